# revision 13
# baseline (speedup 1.0000x reference)
"""Trainium2 Bass kernel: batched 1x1-conv projection + attention-style softmax mixing.

Reference computation (per batch b):
    Wp     = head_w @ W[b]                  # [512, 128]
    scores = Hf[b].T @ Wp                   # [4096, 128]   (Hf = H reshaped [512, 4096])
    A      = softmax(scores, axis=1)        # over M=128
    C      = A @ Wp.T                       # [4096, 512]
    out[b] = C.T                            # [512, 4096] -> [512, 64, 64]

Sharding: data-parallel over batch B=32 across 8 NeuronCores (4 batches/core).

HBM traffic runs fp16 both ways (H cast on host, C output upcast on host);
matmuls take fp16/fp32r operands (1 PE cycle/row), PSUM accumulates fp32.
Measured end-to-end relative error ~1.4e-3 (gate 2e-2).

Schedule notes (from trace analysis):
  - Loads ride the scalar-engine HWDGE ring, stores the sync-engine ring:
    two independent descriptor rings over the 16 DMA engines, no SWDGE ucode
    warm-up. 4KB contiguous HBM runs both directions (2MB macro-tile loads,
    2MB macro-tile stores) keep every DMA engine at its ~26GB/s wire rate.
  - Softmax normalization is deferred: C_raw = E @ WpT on the PE, and the
    1/S multiply happens during PSUM->SBUF evacuation (tensor_mul with the
    matmul-broadcast S reciprocal), split across the vector and pool engines.
    This takes the reciprocal off the PE critical path entirely.
  - The PE stream is software-pipelined one subtile deep: iteration t issues
    scores(t), then sum(t-1)/C(t-1), so exp/reciprocal latency hides under
    the next subtile's score matmuls.
"""

import numpy as np

from concourse import bacc, mybir, tile
from concourse.bass_utils import run_bass_kernel_spmd

B, HD, HH, WW = 32, 512, 64, 64
TD, M = 256, 128
N = HH * WW          # 4096
NCORES = 8
BPC = B // NCORES    # 4 batches per core
NT = 512             # n-tile (free dim per matmul, bounded by one PSUM bank)
NTL = 2048           # n-macro-tile per DMA transfer (4KB strips in HBM)
NMT = N // NTL       # 2 macro-tiles per batch
NSUB = NTL // NT     # 4 matmul subtiles per macro-tile
HC = HD // 128       # 4 h-chunks
SHIFT = 64.0         # softmax stabilization shift

F32 = mybir.dt.float32
F32R = mybir.dt.float32r
F16 = mybir.dt.float16


def build_nc():
    from contextlib import ExitStack

    nc = bacc.Bacc("TRN2", target_bir_lowering=False, debug=False, num_devices=NCORES)
    Hd = nc.dram_tensor("H", [BPC, HD, N], F16, kind="ExternalInput").ap()
    # f32r is bit-identical to f32, so the f32 host arrays upload directly and
    # the weight loads need no SWDGE cast -> they ride the fast HWDGE ring
    Wd = nc.dram_tensor("W", [BPC, TD, M], F32R, kind="ExternalInput").ap()
    hwTd = nc.dram_tensor("head_wT", [TD, HD], F32R, kind="ExternalInput").ap()
    Od = nc.dram_tensor("out", [BPC, HD, N], F16, kind="ExternalOutput").ap()

    with tile.TileContext(nc) as tc, ExitStack() as ctx:
        const = ctx.enter_context(tc.tile_pool(name="const", bufs=1))
        wpool = ctx.enter_context(tc.tile_pool(name="wp", bufs=1))
        hpool = ctx.enter_context(tc.tile_pool(name="h", bufs=4))
        epool = ctx.enter_context(tc.tile_pool(name="e", bufs=2))
        apool = ctx.enter_context(tc.tile_pool(name="a", bufs=3))
        cpool = ctx.enter_context(tc.tile_pool(name="c", bufs=3))
        rpool = ctx.enter_context(tc.tile_pool(name="r", bufs=2))
        ps_sc = ctx.enter_context(tc.tile_pool(name="ps_sc", bufs=2, space="PSUM"))
        ps_c = ctx.enter_context(tc.tile_pool(name="ps_c", bufs=4, space="PSUM"))
        ps_sb = ctx.enter_context(tc.tile_pool(name="ps_sb", bufs=2, space="PSUM"))

        # [128,128] of ones: one matmul turns col-sums of E into S broadcast
        # across all partitions
        ones_f32 = const.tile([128, 128], F32, tag="ones_f32")
        nc.vector.memset(ones_f32[:], 1.0)
        ones_full = const.tile([128, 128], F32R, tag="ones_full")
        nc.vector.tensor_copy(ones_full[:], ones_f32[:])
        neg_shift = const.tile([128, 1], F32, tag="neg_shift")
        nc.vector.memset(neg_shift[:], -SHIFT)

        # Weights go first on the scalar HWDGE ring (1MB total, ~3us); all
        # compute hangs off them so they must not crawl on a cold SWDGE queue.
        hwT = []
        for k in range(2):
            t = const.tile([128, HD], F32R, tag=f"hwT{k}")
            nc.scalar.dma_start(t[:], hwTd[k * 128:(k + 1) * 128, :])
            hwT.append(t)
        # W for all batches as two [128t, b, m] tiles (free dim b*m = 512 so
        # the projection matmuls run at the fp32r 1-cycle/row rate)
        wts = []
        for k in range(2):
            t = wpool.tile([128, BPC, M], F32R, tag=f"wts{k}")
            nc.scalar.dma_start(
                t[:], Wd[:, k * 128:(k + 1) * 128, :].rearrange("b p m -> p b m")
            )
            wts.append(t)

        # --- projections up front (PE is otherwise idle during DMA ramp):
        # wp[j][:, b, :] = (head_w @ W[b]) chunk j   (fp16 lhsT for scores)
        wp_flat = []
        for j in range(HC):
            acc = ps_c.tile([128, BPC * M], F32, tag="c")
            for k in range(2):
                nc.tensor.matmul(
                    acc[:],
                    hwT[k][:, j * 128:(j + 1) * 128],
                    wts[k][:].rearrange("p b m -> p (b m)"),
                    start=(k == 0),
                    stop=(k == 1),
                )
            t = wpool.tile([128, BPC, M], F16, tag=f"wp{j}")
            nc.vector.tensor_copy(t[:].rearrange("p b m -> p (b m)"), acc[:])
            wp_flat.append(t)
        # wpT[b] = Wp[b].T as [128m, 512h]: fp32r copy (lhsT for the raw-E
        # C matmuls, chunks 0-1) and fp16 copy (lhsT for the normalized-A
        # C matmuls, chunks 2-3)
        wpT_all, wpT16_all = [], []
        for b in range(BPC):
            wpT_ps = ps_sc.tile([128, HD], F32, tag="sc")
            for k in range(2):
                nc.tensor.matmul(
                    wpT_ps[:], wts[k][:, b, :], hwT[k][:],
                    start=(k == 0), stop=(k == 1),
                )
            wpT = wpool.tile([128, HD], F32R, tag=f"wpT{b}")
            nc.vector.tensor_copy(wpT[:], wpT_ps[:, 0:HD])
            wpT_all.append(wpT)
            wpT16 = wpool.tile([128, HD], F16, tag=f"wpT16_{b}")
            nc.scalar.copy(wpT16[:], wpT_ps[:, 0:HD])
            wpT16_all.append(wpT16)

        # --- steady state: software-pipelined subtile stream ---
        mtiles = [(b, mt) for b in range(BPC) for mt in range(NMT)]
        subtiles = [(k, s) for k in range(len(mtiles)) for s in range(NSUB)]
        h_tiles = [None] * len(mtiles)

        def load_mtile(k, split):
            b, mt = mtiles[k]
            n0 = mt * NTL
            h = hpool.tile([128, HC, NTL], F16, tag="h")
            if split:
                # first tile: 4 slice loads so subtile 0's matmuls start
                # after 512KB instead of 2MB
                for q in range(NSUB):
                    q0 = q * NT
                    nc.scalar.dma_start(
                        h[:, :, q0:q0 + NT],
                        Hd[b, :, n0 + q0:n0 + q0 + NT].rearrange(
                            "(c p) n -> p c n", p=128),
                    )
            else:
                nc.scalar.dma_start(
                    h[:], Hd[b, :, n0:n0 + NTL].rearrange("(c p) n -> p c n", p=128)
                )
            h_tiles[k] = h

        load_mtile(0, split=True)
        load_mtile(1, split=False)
        load_mtile(2, split=False)

        # Two-deep software pipeline.  Iteration t issues, in PE order:
        #   scores(t) | sum(t-1), C-chunks 0-1 of (t-1) from raw E |
        #   C-chunks 2-3 of (t-2) from normalized A.
        # Chunks 0-1 are normalized during DVE evacuation (tensor_mul by the
        # broadcast 1/S); chunks 2-3 use A = E*r computed on the pool engine
        # (gpsimd cannot touch PSUM) and evacuate via plain scalar copies.
        pend1 = None  # (k, s, e)         awaiting sum/recip/a/C01
        pend2 = None  # (k, s, a, c_tile) awaiting C23 + store
        c_tiles = [None] * len(mtiles)
        for t in range(len(subtiles) + 2):
            if t < len(subtiles):
                k, s = subtiles[t]
                if s == 0 and k + 3 < len(mtiles):
                    load_mtile(k + 3, split=False)
                b, mt = mtiles[k]
                s0 = s * NT
                sc = ps_sc.tile([128, NT], F32, tag="sc")
                for j in range(HC):
                    nc.tensor.matmul(
                        sc[:], wp_flat[j][:, b, :], h_tiles[k][:, j, s0:s0 + NT],
                        start=(j == 0), stop=(j == HC - 1),
                    )
                e = epool.tile([128, NT], F32R, tag="e")
                nc.scalar.activation(
                    e[:], sc[:], mybir.ActivationFunctionType.Exp,
                    bias=neg_shift[:], scale=1.0,
                )
                this1 = (k, s, e)
            else:
                this1 = None

            if pend1 is not None:
                k, s, e = pend1
                b, mt = mtiles[k]
                s0 = s * NT
                if s == 0:
                    c_new = cpool.tile([128, HC, NTL], F16, tag="c_full")
                    c_tiles[k] = c_new
                c_cur = c_tiles[k]
                # S broadcast to every partition in one matmul
                sb = ps_sb.tile([128, NT], F32, tag="sb")
                nc.tensor.matmul(sb[:], ones_full[:], e[:])
                r = rpool.tile([128, NT], F32, tag="r")
                nc.vector.reciprocal_approx_fast(r[:], sb[:])
                a = apool.tile([128, NT], F16, tag="a")
                nc.gpsimd.tensor_mul(a[:], e[:], r[:])
                wpT = wpT_all[b]
                for j in range(2):
                    c_ps = ps_c.tile([128, NT], F32, tag="c")
                    nc.tensor.matmul(c_ps[:], wpT[:, j * 128:(j + 1) * 128], e[:])
                    nc.vector.tensor_mul(c_cur[:, j, s0:s0 + NT], c_ps[:], r[:])
                this2 = (k, s, a, c_cur)
            else:
                this2 = None

            if pend2 is not None:
                k, s, a, c_cur = pend2
                b, mt = mtiles[k]
                s0 = s * NT
                wpT16 = wpT16_all[b]
                for j in range(2, HC):
                    c_ps = ps_c.tile([128, NT], F32, tag="c")
                    nc.tensor.matmul(c_ps[:], wpT16[:, j * 128:(j + 1) * 128], a[:])
                    nc.scalar.copy(c_cur[:, j, s0:s0 + NT], c_ps[:])
                if s == NSUB - 1:
                    n0 = mt * NTL
                    nc.sync.dma_start(
                        Od[b, :, n0:n0 + NTL].rearrange("(c p) n -> p c n", p=128),
                        c_cur[:],
                    )
            pend2 = this2
            pend1 = this1

    nc.compile()
    return nc


_NC = None


def _get_nc():
    global _NC
    if _NC is None:
        _NC = build_nc()
    return _NC


def kernel(H, W, head_w, _run_kwargs=None):
    assert H.shape == (B, HD, HH, WW) and W.shape == (B, TD, M)
    assert head_w.shape == (HD, TD)
    nc = _get_nc()

    Hf = np.ascontiguousarray(H, dtype=np.float32).reshape(B, HD, N).astype(np.float16)
    Wc = np.ascontiguousarray(W, dtype=np.float32)
    hwT = np.ascontiguousarray(head_w.T, dtype=np.float32)

    in_maps = [
        {
            "H": Hf[i * BPC:(i + 1) * BPC],
            "W": Wc[i * BPC:(i + 1) * BPC],
            "head_wT": hwT,
        }
        for i in range(NCORES)
    ]
    res = run_bass_kernel_spmd(
        nc, in_maps, core_ids=list(range(NCORES)), **(_run_kwargs or {})
    )
    out = np.concatenate([res.results[i]["out"] for i in range(NCORES)], axis=0)
    if _run_kwargs:
        kernel.last_results = res
    return out.reshape(B, HD, HH, WW).astype(np.float32)


# revision 17
# speedup vs baseline: 1.0050x; 1.0050x over previous
"""Trainium2 Bass kernel: batched 1x1-conv projection + attention-style softmax mixing.

Reference computation (per batch b):
    Wp     = head_w @ W[b]                  # [512, 128]
    scores = Hf[b].T @ Wp                   # [4096, 128]   (Hf = H reshaped [512, 4096])
    A      = softmax(scores, axis=1)        # over M=128
    C      = A @ Wp.T                       # [4096, 512]
    out[b] = C.T                            # [512, 4096] -> [512, 64, 64]

Sharding: data-parallel over batch B=32 across 8 NeuronCores (4 batches/core).

HBM traffic runs fp16 both ways (H cast on host, C output upcast on host);
matmuls take fp16/fp32r operands (1 PE cycle/row), PSUM accumulates fp32.
Measured end-to-end relative error ~1.4e-3 (gate 2e-2).

Schedule notes (from trace analysis):
  - Loads ride the scalar-engine HWDGE ring, stores the sync-engine ring:
    two independent descriptor rings over the 16 DMA engines, no SWDGE ucode
    warm-up. 4KB contiguous HBM runs both directions (2MB macro-tile loads,
    2MB macro-tile stores) keep every DMA engine at its ~26GB/s wire rate.
  - Softmax normalization is deferred: C_raw = E @ WpT on the PE, and the
    1/S multiply happens during PSUM->SBUF evacuation (tensor_mul with the
    matmul-broadcast S reciprocal), split across the vector and pool engines.
    This takes the reciprocal off the PE critical path entirely.
  - The PE stream is software-pipelined one subtile deep: iteration t issues
    scores(t), then sum(t-1)/C(t-1), so exp/reciprocal latency hides under
    the next subtile's score matmuls.
"""

import numpy as np

from concourse import bacc, mybir, tile
from concourse.bass_utils import run_bass_kernel_spmd

B, HD, HH, WW = 32, 512, 64, 64
TD, M = 256, 128
N = HH * WW          # 4096
NCORES = 8
BPC = B // NCORES    # 4 batches per core
NT = 512             # n-tile (free dim per matmul, bounded by one PSUM bank)
NTL = 2048           # n-macro-tile per DMA transfer (4KB strips in HBM)
NMT = N // NTL       # 2 macro-tiles per batch
NSUB = NTL // NT     # 4 matmul subtiles per macro-tile
HC = HD // 128       # 4 h-chunks
SHIFT = 64.0         # softmax stabilization shift

F32 = mybir.dt.float32
F32R = mybir.dt.float32r
F16 = mybir.dt.float16


def build_nc():
    from contextlib import ExitStack

    nc = bacc.Bacc("TRN2", target_bir_lowering=False, debug=False, num_devices=NCORES)
    Hd = nc.dram_tensor("H", [BPC, HD, N], F16, kind="ExternalInput").ap()
    # fp16 weights (cast on host): halves the cold-start bytes on the ring
    # and lets every projection matmul run at the fp16 1-cycle/row rate
    Wd = nc.dram_tensor("W", [BPC, TD, M], F16, kind="ExternalInput").ap()
    hwTd = nc.dram_tensor("head_wT", [TD, HD], F16, kind="ExternalInput").ap()
    Od = nc.dram_tensor("out", [BPC, HD, N], F16, kind="ExternalOutput").ap()

    with tile.TileContext(nc) as tc, ExitStack() as ctx:
        const = ctx.enter_context(tc.tile_pool(name="const", bufs=1))
        wpool = ctx.enter_context(tc.tile_pool(name="wp", bufs=1))
        hpool = ctx.enter_context(tc.tile_pool(name="h", bufs=4))
        epool = ctx.enter_context(tc.tile_pool(name="e", bufs=2))
        apool = ctx.enter_context(tc.tile_pool(name="a", bufs=3))
        cpool = ctx.enter_context(tc.tile_pool(name="c", bufs=3))
        rpool = ctx.enter_context(tc.tile_pool(name="r", bufs=2))
        ps_sc = ctx.enter_context(tc.tile_pool(name="ps_sc", bufs=2, space="PSUM"))
        ps_c = ctx.enter_context(tc.tile_pool(name="ps_c", bufs=4, space="PSUM"))
        ps_sb = ctx.enter_context(tc.tile_pool(name="ps_sb", bufs=2, space="PSUM"))

        # [128,128] of ones: one matmul turns col-sums of E into S broadcast
        # across all partitions
        ones_f32 = const.tile([128, 128], F32, tag="ones_f32")
        nc.vector.memset(ones_f32[:], 1.0)
        ones_full = const.tile([128, 128], F32R, tag="ones_full")
        nc.vector.tensor_copy(ones_full[:], ones_f32[:])
        neg_shift = const.tile([128, 1], F32, tag="neg_shift")
        nc.vector.memset(neg_shift[:], -SHIFT)

        # Weights go first on the scalar HWDGE ring (0.5MB total); all
        # compute hangs off them so they must not crawl on a cold SWDGE queue.
        hwT = []
        for k in range(2):
            t = const.tile([128, HD], F16, tag=f"hwT{k}")
            nc.scalar.dma_start(t[:], hwTd[k * 128:(k + 1) * 128, :])
            hwT.append(t)
        # W for all batches as two [128t, b, m] tiles
        wts = []
        for k in range(2):
            t = wpool.tile([128, BPC, M], F16, tag=f"wts{k}")
            nc.scalar.dma_start(
                t[:], Wd[:, k * 128:(k + 1) * 128, :].rearrange("b p m -> p b m")
            )
            wts.append(t)

        # --- projections up front (PE is otherwise idle during DMA ramp):
        # wp[j][:, b, :] = (head_w @ W[b]) chunk j   (fp16 lhsT for scores)
        wp_flat = []
        for j in range(HC):
            acc = ps_c.tile([128, BPC * M], F32, tag="c")
            for k in range(2):
                nc.tensor.matmul(
                    acc[:],
                    hwT[k][:, j * 128:(j + 1) * 128],
                    wts[k][:].rearrange("p b m -> p (b m)"),
                    start=(k == 0),
                    stop=(k == 1),
                )
            t = wpool.tile([128, BPC, M], F16, tag=f"wp{j}")
            nc.vector.tensor_copy(t[:].rearrange("p b m -> p (b m)"), acc[:])
            wp_flat.append(t)
        # wpT[b] = Wp[b].T as [128m, 512h]: fp32r copy (lhsT for the raw-E
        # C matmuls, chunks 0-1) and fp16 copy (lhsT for the normalized-A
        # C matmuls, chunks 2-3)
        wpT_all, wpT16_all = [], []
        for b in range(BPC):
            wpT_ps = ps_sc.tile([128, HD], F32, tag="sc")
            for k in range(2):
                nc.tensor.matmul(
                    wpT_ps[:], wts[k][:, b, :], hwT[k][:],
                    start=(k == 0), stop=(k == 1),
                )
            wpT = wpool.tile([128, HD], F32R, tag=f"wpT{b}")
            nc.vector.tensor_copy(wpT[:], wpT_ps[:, 0:HD])
            wpT_all.append(wpT)
            wpT16 = wpool.tile([128, HD], F16, tag=f"wpT16_{b}")
            nc.scalar.copy(wpT16[:], wpT_ps[:, 0:HD])
            wpT16_all.append(wpT16)

        # --- steady state: software-pipelined subtile stream ---
        mtiles = [(b, mt) for b in range(BPC) for mt in range(NMT)]
        subtiles = [(k, s) for k in range(len(mtiles)) for s in range(NSUB)]
        h_tiles = [None] * len(mtiles)

        def load_mtile(k, split):
            b, mt = mtiles[k]
            n0 = mt * NTL
            h = hpool.tile([128, HC, NTL], F16, tag="h")
            if split:
                # first tile: 4 slice loads so subtile 0's matmuls start
                # after 512KB instead of 2MB
                for q in range(NSUB):
                    q0 = q * NT
                    nc.scalar.dma_start(
                        h[:, :, q0:q0 + NT],
                        Hd[b, :, n0 + q0:n0 + q0 + NT].rearrange(
                            "(c p) n -> p c n", p=128),
                    )
            else:
                nc.scalar.dma_start(
                    h[:], Hd[b, :, n0:n0 + NTL].rearrange("(c p) n -> p c n", p=128)
                )
            h_tiles[k] = h

        load_mtile(0, split=True)
        load_mtile(1, split=False)
        load_mtile(2, split=False)

        # Two-deep software pipeline.  Iteration t issues, in PE order:
        #   scores(t) | sum(t-1), C-chunks 0-1 of (t-1) from raw E |
        #   C-chunks 2-3 of (t-2) from normalized A.
        # Chunks 0-1 are normalized during DVE evacuation (tensor_mul by the
        # broadcast 1/S); chunks 2-3 use A = E*r computed on the pool engine
        # (gpsimd cannot touch PSUM) and evacuate via plain scalar copies.
        pend1 = None  # (k, s, e)         awaiting sum/recip/a/C01
        pend2 = None  # (k, s, a, c_tile) awaiting C23 + store
        c_tiles = [None] * len(mtiles)
        for t in range(len(subtiles) + 2):
            if t < len(subtiles):
                k, s = subtiles[t]
                if s == 0 and k + 3 < len(mtiles):
                    load_mtile(k + 3, split=False)
                b, mt = mtiles[k]
                s0 = s * NT
                sc = ps_sc.tile([128, NT], F32, tag="sc")
                for j in range(HC):
                    nc.tensor.matmul(
                        sc[:], wp_flat[j][:, b, :], h_tiles[k][:, j, s0:s0 + NT],
                        start=(j == 0), stop=(j == HC - 1),
                    )
                e = epool.tile([128, NT], F32R, tag="e")
                nc.scalar.activation(
                    e[:], sc[:], mybir.ActivationFunctionType.Exp,
                    bias=neg_shift[:], scale=1.0,
                )
                this1 = (k, s, e)
            else:
                this1 = None

            if pend1 is not None:
                k, s, e = pend1
                b, mt = mtiles[k]
                s0 = s * NT
                if s == 0:
                    c_new = cpool.tile([128, HC, NTL], F16, tag="c_full")
                    c_tiles[k] = c_new
                c_cur = c_tiles[k]
                # S broadcast to every partition in one matmul
                sb = ps_sb.tile([128, NT], F32, tag="sb")
                nc.tensor.matmul(sb[:], ones_full[:], e[:])
                r = rpool.tile([128, NT], F32, tag="r")
                nc.vector.reciprocal_approx_fast(r[:], sb[:])
                a = apool.tile([128, NT], F16, tag="a")
                nc.gpsimd.tensor_mul(a[:], e[:], r[:])
                wpT = wpT_all[b]
                for j in range(2):
                    c_ps = ps_c.tile([128, NT], F32, tag="c")
                    nc.tensor.matmul(c_ps[:], wpT[:, j * 128:(j + 1) * 128], e[:])
                    nc.vector.tensor_mul(c_cur[:, j, s0:s0 + NT], c_ps[:], r[:])
                if s == NSUB - 1:
                    # chunks 0-1 are complete one stage before chunks 2-3:
                    # store them now on the sync ring (chunks 2-3 go on the
                    # SWDGE ring next stage) - two rings double the write rate
                    n0 = mt * NTL
                    nc.sync.dma_start(
                        Od[b, 0:256, n0:n0 + NTL].rearrange(
                            "(c p) n -> p c n", p=128),
                        c_cur[:, 0:2, :],
                    )
                this2 = (k, s, a, c_cur)
            else:
                this2 = None

            if pend2 is not None:
                k, s, a, c_cur = pend2
                b, mt = mtiles[k]
                s0 = s * NT
                wpT16 = wpT16_all[b]
                for j in range(2, HC):
                    c_ps = ps_c.tile([128, NT], F32, tag="c")
                    nc.tensor.matmul(c_ps[:], wpT16[:, j * 128:(j + 1) * 128], a[:])
                    nc.scalar.copy(c_cur[:, j, s0:s0 + NT], c_ps[:])
                if s == NSUB - 1:
                    n0 = mt * NTL
                    nc.gpsimd.dma_start(
                        Od[b, 256:512, n0:n0 + NTL].rearrange(
                            "(c p) n -> p c n", p=128),
                        c_cur[:, 2:4, :],
                    )
            pend2 = this2
            pend1 = this1

    nc.compile()
    return nc


_NC = None


def _get_nc():
    global _NC
    if _NC is None:
        _NC = build_nc()
    return _NC


def kernel(H, W, head_w, _run_kwargs=None):
    assert H.shape == (B, HD, HH, WW) and W.shape == (B, TD, M)
    assert head_w.shape == (HD, TD)
    nc = _get_nc()

    Hf = np.ascontiguousarray(H, dtype=np.float32).reshape(B, HD, N).astype(np.float16)
    Wc = np.ascontiguousarray(W, dtype=np.float32).astype(np.float16)
    hwT = np.ascontiguousarray(head_w.T, dtype=np.float32).astype(np.float16)

    in_maps = [
        {
            "H": Hf[i * BPC:(i + 1) * BPC],
            "W": Wc[i * BPC:(i + 1) * BPC],
            "head_wT": hwT,
        }
        for i in range(NCORES)
    ]
    res = run_bass_kernel_spmd(
        nc, in_maps, core_ids=list(range(NCORES)), **(_run_kwargs or {})
    )
    out = np.concatenate([res.results[i]["out"] for i in range(NCORES)], axis=0)
    if _run_kwargs:
        kernel.last_results = res
    return out.reshape(B, HD, HH, WW).astype(np.float32)
